# revision 30
# baseline (speedup 1.0000x reference)
"""Trainium2 Bass kernel for a single causal attention head (with the
faithful source bug: q = x @ W_key, W_query unused).

Full-input contract: kernel(x, W_key, W_query, W_value) -> [8, 2048, 128].
Sharding: data-parallel over batch B=8 across 8 NeuronCores (1 batch/core).

Per-core math (T=2048, C=1024, H=128):
    K = x @ W_key            (V = x @ W_value)
    S = K @ K.T * H**-0.5    (symmetric since q == k)
    out = softmax(causal(S)) @ V

v3 design (profile-driven rebuild of the original kernel):
  - PE warm-up matmuls during the ~10us input-DMA window beat the
    p-state ramp so real matmuls start at full clock.
  - Projections contract over C in PSUM producing KT/VT [h, t]; K of
    the last chunk prioritized and V of the last chunk deferred so the
    kt PSUM->SBUF casts (critical path to scores) start immediately
    after the last x byte. kt casts on DVE, vt casts split DVE/ACT.
  - V^T -> per-tile V [t, h] via ONE whole-tensor XBAR DMA transpose
    (validated on HW), then a strided copy into the 129-wide vaug
    whose last column is ones — the ones ride the AV matmul so the
    softmax denominator accumulates for free (the v1 trick).
  - Scores: upper triangle only (S symmetric). exp on ACT in 1024-wide
    PSUM tiles (24 ACTIVATEs instead of 40 — ACT fixed costs are the
    mid-phase critical path), diag tile exp'd unmasked and masked
    after in fp16 (cheap 2-byte DVE op off the PE->ACT chain).
  - AV per (j, i): stationary E_ji [k, q in tile i], moving
    vaug_j [k, 129]; av column i accumulates in its own PSUM bank.
    Catch-up schedule front-loads early rounds (ACT-bound) so the PE
    has slack when rounds get PE-bound near the causal triangle base.
  - Outputs batch 4 seq tiles per DMA (4 output DMAs instead of 16 —
    DIRECT2D triggers cost ~0.7us of engine time each).
"""

import numpy as np

import concourse.bass as bass
import concourse.mybir as mybir
import concourse.tile as tile
from concourse import bacc, bass_utils


P = 128
T = 2048
C = 1024
H = 128
NT = T // P  # 16 seq tiles
NC = C // P  # 8 contraction tiles
NCORES = 8
NAV = P + 1  # v | ones
SCALE = float(H) ** -0.5
F32 = mybir.dt.float32
FP16 = mybir.dt.float16
EXP = mybir.ActivationFunctionType.Exp

CHW = 512
CHN = T // CHW  # 4 chunks of the t axis during projections
NWARM = 6  # PE warm-up matmuls (p-state ramp) during the input DMA


def av_open_round(i):
    """First round column i may issue AV updates. Columns pack three
    per PSUM bank (bank i//3 pre-zeroed, start=False accumulation);
    columns 12-15 wait for their bank's re-zero after the previous
    tenants drain."""
    if i <= 11:
        return max(1, i - 6)
    if i <= 14:  # bank 0 re-zeroed at round 4 (cols 0-2 drained by r3)
        return max(5, i - 6)
    return 8  # bank 1 re-zeroed at round 7 (cols 3-5 drained by r6)


def build_av_schedule():
    """AV update (j, i) -> round, spread evenly over the column's open
    rounds. Safe when j <= round-1; j == round only stalls briefly on
    the concurrent exp."""
    sched = {}
    for i in range(NT):
        rounds = list(range(av_open_round(i), i + 1)) or [av_open_round(i)]
        js = list(range(i + 1))
        k = len(rounds)
        base, rem = divmod(len(js), k)
        sizes = [base + (1 if x < rem else 0) for x in range(k)]
        pos = 0
        for r, sz in zip(rounds, sizes):
            for j in js[pos : pos + sz]:
                sched.setdefault(r, []).append((j, i))
            pos += sz
    return sched


AV_SCHED = build_av_schedule()
REG = 160  # f32 region stride inside an AV bank (3 x 129-wide columns)


def build_module():
    nc = bacc.Bacc(
        "TRN2", target_bir_lowering=False, debug=False, num_devices=NCORES
    )
    xT_d = nc.dram_tensor("xT", [C, T], FP16, kind="ExternalInput").ap()
    # weights arranged [p, kv, c, h] on the host (one fused DMA)
    w_d = nc.dram_tensor("W", [P, 2, NC, H], FP16, kind="ExternalInput").ap()
    # [:, 0] upper-tri-incl-diag mask [k, q]; [:, 1] identity
    cons_d = nc.dram_tensor("CONS", [P, 2, P], FP16, kind="ExternalInput").ap()
    y_d = nc.dram_tensor("y", [T, H], F32, kind="ExternalOutput").ap()

    # offsets of score row-block j inside e_all (block j holds queries
    # b in [j*128, 2048) -> width (NT-j)*128)
    offs = []
    off = 0
    for j in range(NT):
        offs.append(off)
        off += (NT - j) * P
    e_width = off  # 136 * 128 = 17408

    def rw(j):  # row block width
        return (NT - j) * P

    with tile.TileContext(nc) as tc:
        with (
            tc.tile_pool(name="const", bufs=1) as const,
            tc.tile_pool(name="xt", bufs=8) as xt_pool,
            tc.tile_pool(name="kv", bufs=1) as kv,
            tc.tile_pool(name="e", bufs=1) as e_pool,
            tc.tile_pool(name="ysb", bufs=2) as ysb_pool,
            tc.tile_pool(name="rcp", bufs=4) as rcp_pool,
        ):
            w_sb = const.tile([P, 2, NC, H], FP16)
            nc.scalar.dma_start(w_sb[:], w_d[:])
            wk_sb = w_sb[:, 0]
            wv_sb = w_sb[:, 1]
            cons = const.tile([P, 2, P], FP16)

            # input x chunks: triggers alternate between the two HWDGE
            # sequencers (sync / scalar). W rides the scalar queue so
            # xT0 (which gates the first projection matmul) is not
            # queued behind it on sync.
            xts = [
                xt_pool.tile([P, T], FP16, tag="xt", name=f"xt{c}")
                for c in range(NC)
            ]
            nc.sync.dma_start(xts[0][:], xT_d[0:P, :])
            for c in range(1, NC):
                eng = nc.scalar if c % 2 else nc.sync
                eng.dma_start(xts[c][:], xT_d[c * P : (c + 1) * P, :])
            nc.sync.dma_start(cons[:], cons_d[:])
            umask = cons[:, 0]

            # pre-warm the ACT exp table during the input DMAs
            warm = const.tile([P, 1], F32)
            nc.vector.memset(warm[:], 0.0)
            nc.scalar.activation(warm[:], warm[:], EXP)

            kt_r = kv.tile([P, T], FP16)  # K^T [h, t]
            vt_sb = kv.tile([P, T], FP16)  # V^T [h, t]
            vtmp = kv.tile([P, NT, P], FP16)  # XBAR dst: V [t, h] tiles
            vaug = kv.tile([P, NT, NAV], FP16)  # [v | ones]
            nc.vector.memset(vaug[:], 1.0)  # ones cols; v overwritten
            e_all = e_pool.tile([P, e_width], FP16)

            with tc.tile_pool(name="psproj", bufs=8, space="PSUM") as psp:
                # PE warm-up: garbage matmuls to exit the low p-states
                # while the x DMA streams in.
                trash = const.tile([P, CHW], FP16)
                nc.vector.memset(trash[:], 0.0)
                warm_ps = psp.tile([P, CHW], F32, tag="ps", name="warmps")
                for _ in range(NWARM):
                    nc.tensor.matmul(
                        warm_ps[:],
                        trash[:, 0:P],
                        trash[:],
                        start=True,
                        stop=True,
                    )

                kt_ps = [
                    psp.tile([P, CHW], F32, tag="ps", name=f"ktps{ch}")
                    for ch in range(CHN)
                ]
                vt_ps = [
                    psp.tile([P, CHW], F32, tag="ps", name=f"vtps{ch}")
                    for ch in range(CHN)
                ]
                # K first within each chunk; V of the last chunk deferred
                # so the kt casts (critical for scores) start earliest.
                for c in range(NC):
                    for ch in range(CHN):
                        rhs = xts[c][:, ch * CHW : (ch + 1) * CHW]
                        nc.tensor.matmul(
                            kt_ps[ch][:],
                            wk_sb[:, c, :],
                            rhs,
                            start=(c == 0),
                            stop=(c == NC - 1),
                        )
                    if c < NC - 1:
                        for ch in range(CHN):
                            rhs = xts[c][:, ch * CHW : (ch + 1) * CHW]
                            nc.tensor.matmul(
                                vt_ps[ch][:],
                                wv_sb[:, c, :],
                                rhs,
                                start=(c == 0),
                                stop=False,
                            )
                # kt -> SBUF fp16 (DVE), in score-consumption order
                for ch in range(CHN):
                    sl = slice(ch * CHW, (ch + 1) * CHW)
                    nc.vector.tensor_copy(kt_r[:, sl], kt_ps[ch][:])
                # deferred V of the last chunk, then vt casts + XBAR
                for ch in range(CHN):
                    rhs = xts[NC - 1][:, ch * CHW : (ch + 1) * CHW]
                    nc.tensor.matmul(
                        vt_ps[ch][:],
                        wv_sb[:, NC - 1, :],
                        rhs,
                        start=False,
                        stop=True,
                    )
                # vt casts split: ACT takes 2 (idle pre-exp), DVE 2.
                # V^T [h, t] -> per-tile V [t, h] via the DMA crossbar,
                # in two halves so the first fires as soon as the ACT
                # casts land (the XBAR->vaug chain gates the first AV).
                nc.scalar.copy(vt_sb[:, 0:CHW], vt_ps[0][:])
                nc.scalar.copy(vt_sb[:, CHW : 2 * CHW], vt_ps[1][:])
                nc.sync.dma_start_transpose(
                    vtmp[:, 0 : NT // 2, :], vt_sb[:, 0 : T // 2]
                )
                nc.vector.tensor_copy(vt_sb[:, 2 * CHW : 3 * CHW], vt_ps[2][:])
                nc.vector.tensor_copy(vt_sb[:, 3 * CHW : 4 * CHW], vt_ps[3][:])
                nc.sync.dma_start_transpose(
                    vtmp[:, NT // 2 : NT, :], vt_sb[:, T // 2 : T]
                )

            with (
                tc.tile_pool(name="pssc", bufs=2, space="PSUM") as pssc,
                tc.tile_pool(name="psav", bufs=4, space="PSUM") as psav,
            ):
                # four persistent AV banks, three 129-wide column
                # regions each (640B stride); pre-zeroed, start=False
                # accumulation, so the one-group-per-bank rule never
                # applies. Cols 0-11 use fresh regions; 12-15 reuse
                # banks 0/1 after an inter-generation re-zero.
                av_banks = [
                    psav.tile([P, 512], F32, tag="ps", name=f"avb{b}")
                    for b in range(4)
                ]
                nc.scalar.memzero(av_banks[0][:])
                nc.scalar.memzero(av_banks[1][:])
                nc.vector.memset(av_banks[2][:], 0.0)
                nc.vector.memset(av_banks[3][:], 0.0)

                def av_region(i):
                    b, reg = (i // 3, i % 3) if i <= 11 else (
                        (0, i - 12) if i <= 14 else (1, 0)
                    )
                    return av_banks[b][:, REG * reg : REG * reg + NAV]

                y_tiles = {}

                def scores_row(j):
                    """Score matmuls + exp for row block j in 1024-wide
                    PSUM tiles (2 matmuls, 1 ACTIVATE per tile)."""
                    b0 = j * P
                    width = rw(j)
                    pos = 0
                    while pos < width:
                        wt = min(1024, width - pos)
                        s_ps = pssc.tile(
                            [P, 1024], F32, tag="ps", name=f"sps{j}_{pos}"
                        )
                        p2 = 0
                        while p2 < wt:
                            w2 = min(CHW, wt - p2)
                            nc.tensor.matmul(
                                s_ps[:, p2 : p2 + w2],
                                kt_r[:, b0 : b0 + P],
                                kt_r[:, b0 + pos + p2 : b0 + pos + p2 + w2],
                                start=True,
                                stop=True,
                            )
                            p2 += w2
                        nc.scalar.activation(
                            e_all[:, offs[j] + pos : offs[j] + pos + wt],
                            s_ps[:, :wt],
                            EXP,
                            scale=SCALE,
                        )
                        pos += wt

                def mask_row(j):
                    # causal mask on the diag tile (post-exp, fp16)
                    nc.vector.tensor_mul(
                        e_all[:, offs[j] : offs[j] + P],
                        e_all[:, offs[j] : offs[j] + P],
                        umask[:],
                    )

                def av_update(j, i):
                    eji = e_all[
                        :, offs[j] + (i - j) * P : offs[j] + (i - j + 1) * P
                    ]
                    nc.tensor.matmul(
                        av_region(i),
                        eji,
                        vaug[:, j, :],
                        start=False,
                        stop=(j == i),
                        skip_group_check=True,
                    )

                def drain_col(i):
                    """Column i closed last round: normalize by the
                    ones-column denominator into the group y tile."""
                    g = i // 4
                    if i % 4 == 0:
                        y_tiles[g] = ysb_pool.tile(
                            [P, 4, P], F32, tag="ysb", name=f"ysb{g}"
                        )
                    av = av_region(i)
                    recip = rcp_pool.tile(
                        [P, 1], F32, tag="recip", name=f"rcp{i}"
                    )
                    nc.vector.reciprocal(recip[:], av[:, P : P + 1])
                    nc.vector.tensor_scalar_mul(
                        y_tiles[g][:, i % 4, :], av[:, 0:P], recip[:]
                    )
                    if i % 4 == 3:
                        y_view = y_d[512 * g : 512 * (g + 1), :].rearrange(
                            "(i p) h -> p i h", p=P
                        )
                        nc.sync.dma_start(y_view, y_tiles[g][:])

                # vaug = [vtmp | ones]: per-tile contiguous copies once
                # the XBAR lands (strided full-tensor copy miscompiles),
                # emitted incrementally a few tiles ahead of AV use.
                vaug_done = [0]

                def vaug_copy_upto(n):
                    while vaug_done[0] < min(n, NT):
                        j = vaug_done[0]
                        nc.vector.tensor_copy(
                            vaug[:, j, 0:P], vtmp[:, j, :]
                        )
                        vaug_done[0] += 1

                # software pipeline: round r issues scores row r+1,
                # then the AV catch-up batch for live columns.
                scores_row(0)
                for r in range(NT + 2):
                    # drain first: column i closes at round max(1, i),
                    # and its PSUM slot is re-allocated 4 columns later —
                    # the drain reads must be emitted before the new
                    # column's first write for Tile to see the WAR.
                    if r == 2:
                        drain_col(0)
                    if 2 <= r <= NT:
                        drain_col(r - 1)
                    # re-zero banks between column generations (after
                    # the previous tenants' drain reads are emitted)
                    if r == 4:
                        nc.vector.memset(av_banks[0][:], 0.0)
                    if r == 7:
                        nc.vector.memset(av_banks[1][:], 0.0)
                    # mask before any AV update may read row r's diag
                    if r < NT:
                        mask_row(r)
                    vaug_copy_upto(4 + 2 * r)
                    batch = AV_SCHED.get(r, ())
                    if r > 1:
                        for j, i in batch[:2]:
                            av_update(j, i)
                        batch = batch[2:]
                    if r + 1 < NT:
                        scores_row(r + 1)
                    for j, i in batch:
                        av_update(j, i)

    nc.compile()
    return nc


_NC_CACHE = None


def _get_module():
    global _NC_CACHE
    if _NC_CACHE is None:
        _NC_CACHE = build_module()
    return _NC_CACHE


def run(in_maps, trace=False, **kw):
    nc = _get_module()
    return bass_utils.run_bass_kernel_spmd(
        nc, in_maps, core_ids=list(range(NCORES)), trace=trace, **kw
    )


def make_in_maps(x, W_key, W_value):
    x = np.asarray(x, dtype=np.float32).astype(np.float16)
    xT = np.ascontiguousarray(x.transpose(0, 2, 1))
    wk = np.asarray(W_key, np.float32).astype(np.float16)
    wk = wk.reshape(NC, P, H).transpose(1, 0, 2)
    wv = np.asarray(W_value, np.float32).astype(np.float16)
    wv = wv.reshape(NC, P, H).transpose(1, 0, 2)
    w = np.ascontiguousarray(np.stack([wk, wv], axis=1))  # [P, 2, NC, H]
    umask = np.triu(np.ones((P, P), dtype=np.float16))  # keep q >= k
    ident = np.eye(P, dtype=np.float16)
    cons = np.ascontiguousarray(np.stack([umask, ident], axis=1))
    return [{"xT": xT[b], "W": w, "CONS": cons} for b in range(NCORES)]


def kernel(x, W_key, W_query, W_value):
    # W_query intentionally unused: the reference applies W_key for q too.
    del W_query
    res = run(make_in_maps(x, W_key, W_value), trace=False)
    return np.stack([res.results[b]["y"] for b in range(NCORES)], axis=0)


# revision 35
# speedup vs baseline: 1.1518x; 1.1518x over previous
"""Trainium2 Bass kernel for a single causal attention head (with the
faithful source bug: q = x @ W_key, W_query unused).

Full-input contract: kernel(x, W_key, W_query, W_value) -> [8, 2048, 128].
Sharding: data-parallel over batch B=8 across 8 NeuronCores (1 batch/core).

Per-core math (T=2048, C=1024, H=128):
    K = x @ W_key            (V = x @ W_value)
    S = K @ K.T * H**-0.5    (symmetric since q == k)
    out = softmax(causal(S)) @ V

v5 design (profile-driven; the ACT engine's exp stream is the
mid-phase critical path, so everything is organized to start it as
early as possible and keep it gapless):
  - Projections split by T-halves: K for queries/keys [0,1024) only
    needs the first half of the x DMA, so scores+exp for the top-left
    triangle start ~4us after half the input landed, overlapping the
    rest of the DMA and the second projection half. Phase A = rows
    0-7 queries [128j,1024); phase B = rows 0-7 queries [1024,2048)
    then rows 8-15.
  - exp in <=1024-wide PSUM pieces (24 ACTIVATEs), diag tiles exp'd
    unmasked and masked post-hoc in fp16 (off the PE->ACT chain).
  - AV per (j, i): stationary E_ji [k, q in tile i], moving
    vaug_j [k, v|ones] — denominators ride for free; back-to-back AV
    matmuls sustain ~57ns (LDWEIGHTS pipelined), so AV is cheap.
    Columns pack three per PSUM bank (640B regions, pre-zeroed,
    start=False accumulation) so 12+ columns are concurrently open
    and the work spreads across all rounds instead of piling after
    the last exp.
  - V^T -> per-tile V [t, h] via per-half XBAR DMA transposes.
  - Outputs batch 4 seq tiles per DMA. Input DMA triggers balanced
    across the two HWDGE queues so xT chunk 0 is never queued behind
    the weights.
"""

import numpy as np

import concourse.bass as bass
import concourse.mybir as mybir
import concourse.tile as tile
from concourse import bacc, bass_utils


P = 128
T = 2048
C = 1024
H = 128
NT = T // P  # 16 seq tiles
NC = C // P  # 8 contraction tiles
NCORES = 8
NAV = P + 1  # v | ones
HT = T // 2  # half of t
SCALE = float(H) ** -0.5
F32 = mybir.dt.float32
FP16 = mybir.dt.float16
EXP = mybir.ActivationFunctionType.Exp

CHW = 512
NWARM = 2
NR = 25  # rounds: R0-7 phase A, R8-23 phase B, R24 final drains
REG = 160  # f32 region stride inside an AV bank (3 x 129-wide columns)


# AV column -> rounds window. Columns pack 3/bank; banks become free
# progressively (they reuse projection-psum slots after the casts):
# bank0/1 (cols 0-5) at R5, bank2 (cols 6-8) at R6(+1), bank3
# (cols 9-11) at R7(+2); gen-1: cols 12-14 on bank0 after R9 re-zero,
# col 15 on bank1 after R10. Cols >= 8 also need phase-B pieces:
# update (j, i) valid at R >= j+8.
AV_ROUNDS = {
    0: [5], 1: [5, 6], 2: [5, 6], 3: [5, 6, 7], 4: [5, 6, 7],
    5: [5, 6, 7], 6: [7, 8], 7: [7, 8, 9],
    8: list(range(9, 17)), 9: list(range(9, 18)),
    10: list(range(9, 19)), 11: list(range(9, 20)),
    12: list(range(10, 21)), 13: list(range(10, 22)),
    14: list(range(10, 23)), 15: list(range(11, 24)),
}


def build_av_schedule():
    sched = {}
    for i in range(NT):
        rounds = AV_ROUNDS[i]
        js = list(range(i + 1))
        k = len(rounds)
        base, rem = divmod(len(js), k)
        sizes = [base + (1 if x < rem else 0) for x in range(k)]
        pos = 0
        for r, sz in zip(rounds, sizes):
            for j in js[pos : pos + sz]:
                if i >= 8:
                    assert j <= r - 8, (i, j, r)
                sched.setdefault(r, []).append((j, i))
            pos += sz
    return sched


AV_SCHED = build_av_schedule()
# drain rounds (column close = last round of AV_ROUNDS, +1, 2/round)
DRAINS = {
    7: [0, 1], 8: [2, 3], 9: [4, 5], 10: [6], 11: [7],
    17: [8], 18: [9], 19: [10], 20: [11],
    21: [12], 22: [13], 23: [14], 24: [15],
}
YDMA = {8: 0, 11: 1, 20: 2, 24: 3}  # round -> y group


def build_module():
    nc = bacc.Bacc(
        "TRN2", target_bir_lowering=False, debug=False, num_devices=NCORES
    )
    xT_d = nc.dram_tensor("xT", [C, T], FP16, kind="ExternalInput").ap()
    w_d = nc.dram_tensor("W", [P, 2, NC, H], FP16, kind="ExternalInput").ap()
    cons_d = nc.dram_tensor("CONS", [P, 2, P], FP16, kind="ExternalInput").ap()
    y_d = nc.dram_tensor("y", [T, H], F32, kind="ExternalOutput").ap()

    offs = []
    off = 0
    for j in range(NT):
        offs.append(off)
        off += (NT - j) * P
    e_width = off  # 17408

    def rw(j):
        return (NT - j) * P

    with tile.TileContext(nc) as tc:
        with (
            tc.tile_pool(name="const", bufs=1) as const,
            tc.tile_pool(name="xt", bufs=8) as xt_pool,
            tc.tile_pool(name="kv", bufs=1) as kv,
            tc.tile_pool(name="e", bufs=1) as e_pool,
            tc.tile_pool(name="ysb", bufs=2) as ysb_pool,
            tc.tile_pool(name="rcp", bufs=4) as rcp_pool,
        ):
            w_sb = const.tile([P, 2, NC, H], FP16)
            nc.scalar.dma_start(w_sb[:], w_d[:])
            wk_sb = w_sb[:, 0]
            wv_sb = w_sb[:, 1]
            cons = const.tile([P, 2, P], FP16)

            # x half-chunks: [c-block, t-half]; half 0 first on both
            # queues (W leads the scalar queue)
            xts = [
                xt_pool.tile([P, T], FP16, tag="xt", name=f"xt{c}")
                for c in range(NC)
            ]
            for h in range(2):
                for c in range(NC):
                    eng = nc.scalar if c % 2 else nc.sync
                    eng.dma_start(
                        xts[c][:, h * HT : (h + 1) * HT],
                        xT_d[c * P : (c + 1) * P, h * HT : (h + 1) * HT],
                    )
            nc.sync.dma_start(cons[:], cons_d[:])
            umask = cons[:, 0]

            warm = const.tile([P, 1], F32)
            nc.vector.memset(warm[:], 0.0)
            nc.scalar.activation(warm[:], warm[:], EXP)

            kt_r = kv.tile([P, T], FP16)  # K^T [h, t]
            vt_sb = kv.tile([P, T], FP16)  # V^T [h, t]
            vtmp = kv.tile([P, NT, P], FP16)  # XBAR dst: V [t, h] tiles
            vaug = kv.tile([P, NT, NAV], FP16)  # [v | ones]
            nc.vector.memset(vaug[:], 1.0)
            e_all = e_pool.tile([P, e_width], FP16)

            with (
                tc.tile_pool(name="psproj", bufs=4, space="PSUM") as psp,
                tc.tile_pool(name="pssc", bufs=2, space="PSUM") as pssc,
            ):
                # --- static PSUM plan: 4 proj/AV banks + 4 score banks.
                # Half-1 projection tiles reuse half-0's slots (freed by
                # the casts), and the 4 AV banks reuse the proj slots
                # after the half-1 casts. Warm-ups write into the first
                # kt half-0 bank before its real start=True matmul.
                trash = const.tile([P, CHW], FP16)
                nc.vector.memset(trash[:], 0.0)
                proj_ps = {}
                for h in range(2):
                    for kvi in range(2):
                        proj_ps[(h, kvi)] = [
                            psp.tile(
                                [P, CHW], F32, tag="ps", name=f"pj{h}{kvi}{ch}"
                            )
                            for ch in range(2)
                        ]

                for _ in range(NWARM):
                    nc.tensor.matmul(
                        proj_ps[(0, 0)][0][:], trash[:, 0:P], trash[:],
                        start=True, stop=True, skip_group_check=True,
                    )

                def proj_mms(h, kvi, cs):
                    w_src = wk_sb if kvi == 0 else wv_sb
                    for c in cs:
                        for ch in range(2):
                            lo = h * HT + ch * CHW
                            nc.tensor.matmul(
                                proj_ps[(h, kvi)][ch][:],
                                w_src[:, c, :],
                                xts[c][:, lo : lo + CHW],
                                start=(c == 0),
                                stop=(c == NC - 1),
                            )

                def cast_half(h, kvi, engines):
                    dst = kt_r if kvi == 0 else vt_sb
                    for ch in range(2):
                        lo = h * HT + ch * CHW
                        eng = engines[ch]
                        if eng is nc.scalar:
                            nc.scalar.copy(
                                dst[:, lo : lo + CHW], proj_ps[(h, kvi)][ch][:]
                            )
                        else:
                            nc.vector.tensor_copy(
                                dst[:, lo : lo + CHW], proj_ps[(h, kvi)][ch][:]
                            )

                # --- AV banks: 3 columns per bank, memset + start=False
                # accumulation; allocated progressively as the proj
                # slots they reuse are freed by the casts.
                av_banks = {}

                def av_bank_open(b):
                    av_banks[b] = psp.tile(
                        [P, 512], F32, tag="ps", name=f"avb{b}"
                    )
                    nc.vector.memset(av_banks[b][:], 0.0)

                def av_region(i):
                    b, reg = (i // 3, i % 3) if i <= 11 else (
                        (0, i - 12) if i <= 14 else (1, 0)
                    )
                    return av_banks[b][:, REG * reg : REG * reg + NAV]

                # --- scores / exp pieces ------------------------------
                def spiece(j, part):
                    """part 0: A piece (queries [128j, 1024) for j<8,
                    whole row for j>=8); part 1: B piece [1024, 2048)."""
                    if j < 8:
                        wA = 1024 - j * P
                        base, wt = (0, wA) if part == 0 else (wA, 1024)
                    else:
                        base, wt = 0, rw(j)
                    b0 = j * P
                    s_ps = pssc.tile(
                        [P, 1024], F32, tag="ps", name=f"sp{j}_{part}"
                    )
                    p2 = 0
                    while p2 < wt:
                        w2 = min(CHW, wt - p2)
                        nc.tensor.matmul(
                            s_ps[:, p2 : p2 + w2],
                            kt_r[:, b0 : b0 + P],
                            kt_r[:, b0 + base + p2 : b0 + base + p2 + w2],
                            start=True,
                            stop=True,
                        )
                        p2 += w2
                    nc.scalar.activation(
                        e_all[:, offs[j] + base : offs[j] + base + wt],
                        s_ps[:, :wt],
                        EXP,
                        scale=SCALE,
                    )

                def mask_row(j):
                    nc.vector.tensor_mul(
                        e_all[:, offs[j] : offs[j] + P],
                        e_all[:, offs[j] : offs[j] + P],
                        umask[:],
                    )

                def av_update(j, i):
                    eji = e_all[
                        :, offs[j] + (i - j) * P : offs[j] + (i - j + 1) * P
                    ]
                    nc.tensor.matmul(
                        av_region(i),
                        eji,
                        vaug[:, j, :],
                        start=False,
                        stop=(j == i),
                        skip_group_check=True,
                    )

                y_tiles = {}

                def drain_col(i):
                    g = i // 4
                    if i % 4 == 0:
                        y_tiles[g] = ysb_pool.tile(
                            [P, 4, P], F32, tag="ysb", name=f"ysb{g}"
                        )
                    av = av_region(i)
                    recip = rcp_pool.tile(
                        [P, 1], F32, tag="recip", name=f"rcp{i}"
                    )
                    nc.vector.reciprocal(recip[:], av[:, P : P + 1])
                    nc.vector.tensor_scalar_mul(
                        y_tiles[g][:, i % 4, :], av[:, 0:P], recip[:]
                    )

                def y_out(g):
                    y_view = y_d[512 * g : 512 * (g + 1), :].rearrange(
                        "(i p) h -> p i h", p=P
                    )
                    nc.sync.dma_start(y_view, y_tiles[g][:])

                vaug_done = [0]

                def vaug_copy_upto(n):
                    while vaug_done[0] < min(n, NT):
                        j = vaug_done[0]
                        nc.vector.tensor_copy(
                            vaug[:, j, 0:P], vtmp[:, j, :]
                        )
                        vaug_done[0] += 1

                # --- pre-round emission -------------------------------
                proj_mms(0, 0, range(NC))  # K half-0
                cast_half(0, 0, (nc.scalar, nc.vector))  # kt h0
                spiece(0, 0)  # scores row 0 piece A -> first exp

                # --- round loop ---------------------------------------
                for r in range(NR):
                    for i in DRAINS.get(r, ()):
                        drain_col(i)
                    # masks: rows 0-7 at R0-7, rows 8-15 at R16-23
                    if r < 8:
                        mask_row(r)
                    elif 16 <= r < 24:
                        mask_row(r - 8)
                    # scores pieces: A rows 1-7 at R0-6; B row rb at
                    # R7+rb; rows 8-15 at R15-22
                    if r < 7:
                        spiece(r + 1, 0)
                    elif r < 15:
                        spiece(r - 7, 1)
                    elif r < 23:
                        spiece(r - 7, 0)
                    # V half-0 early (weights land with the half-0 DMA)
                    if r == 0:
                        proj_mms(0, 1, range(0, 4))
                    if r == 1:
                        proj_mms(0, 1, range(4, NC))
                        cast_half(0, 1, (nc.vector, nc.vector))
                        nc.sync.dma_start_transpose(
                            vtmp[:, 0:8, :], vt_sb[:, 0:HT]
                        )
                    # K half-1 as its chunks land; casts feed phase B
                    if 1 <= r <= 4:
                        proj_mms(1, 0, range(2 * (r - 1), 2 * r))
                    if r == 4:
                        cast_half(1, 0, (nc.scalar, nc.vector))
                    # V half-1 after K half-1
                    if r == 5:
                        proj_mms(1, 1, range(0, 4))
                        av_bank_open(0)  # reuses kt_h1 slots (cast @R4)
                        av_bank_open(1)
                    if r == 6:
                        proj_mms(1, 1, range(4, NC))
                        cast_half(1, 1, (nc.vector, nc.vector))
                        nc.sync.dma_start_transpose(
                            vtmp[:, 8:16, :], vt_sb[:, HT:T]
                        )
                    if r == 7:
                        av_bank_open(2)  # reuse vt_h1 slots (cast @R6)
                        av_bank_open(3)
                    # inter-generation AV bank re-zeros
                    if r == 9:
                        nc.vector.memset(av_banks[0][:], 0.0)
                    if r == 10:
                        nc.vector.memset(av_banks[1][:], 0.0)
                    # vaug tiles: 0-7 after XBAR-A, 8-15 after XBAR-B
                    if r >= 2:
                        vaug_copy_upto(8 if r < 7 else 8 + 2 * (r - 6))
                    for j, i in AV_SCHED.get(r, ()):
                        av_update(j, i)
                    if r in YDMA:
                        y_out(YDMA[r])

    nc.compile()
    return nc


_NC_CACHE = None


def _get_module():
    global _NC_CACHE
    if _NC_CACHE is None:
        _NC_CACHE = build_module()
    return _NC_CACHE


def run(in_maps, trace=False, **kw):
    nc = _get_module()
    return bass_utils.run_bass_kernel_spmd(
        nc, in_maps, core_ids=list(range(NCORES)), trace=trace, **kw
    )


def make_in_maps(x, W_key, W_value):
    x = np.asarray(x, dtype=np.float32).astype(np.float16)
    xT = np.ascontiguousarray(x.transpose(0, 2, 1))
    wk = np.asarray(W_key, np.float32).astype(np.float16)
    wk = wk.reshape(NC, P, H).transpose(1, 0, 2)
    wv = np.asarray(W_value, np.float32).astype(np.float16)
    wv = wv.reshape(NC, P, H).transpose(1, 0, 2)
    w = np.ascontiguousarray(np.stack([wk, wv], axis=1))  # [P, 2, NC, H]
    umask = np.triu(np.ones((P, P), dtype=np.float16))  # keep q >= k
    ident = np.eye(P, dtype=np.float16)
    cons = np.ascontiguousarray(np.stack([umask, ident], axis=1))
    return [{"xT": xT[b], "W": w, "CONS": cons} for b in range(NCORES)]


def kernel(x, W_key, W_query, W_value):
    # W_query intentionally unused: the reference applies W_key for q too.
    del W_query
    res = run(make_in_maps(x, W_key, W_value), trace=False)
    return np.stack([res.results[b]["y"] for b in range(NCORES)], axis=0)
